# revision 22
# baseline (speedup 1.0000x reference)
"""Trainium2 Bass kernel for nn_CompressiveMemory_57750130262084.

The reference computes (B=8, S=4096, DK=DV=1024):
    sigma  = elu(query) + 1                                  [B,S,DK]
    memory = einsum('bkd,bsv->bkv', swap(sigma), value)      [B,DK,DV]
    z_norm = sum_s sigma                                     [B,DK]
    out    = einsum('bsd,bkv->bsv', sigma, memory)
           / einsum('bsd,bk->bs',  sigma, z_norm)[..., None]

Every einsum uses disjoint summed subscripts, so each factorises into
outer products of independent reductions:
    memory[b,k,v]    = z_norm[b,k] * VS[b,v]      with VS[b,v] = sum_s value[b,s,v]
    retrieved[b,s,v] = rs[b,s] * Z[b] * VS[b,v]   with rs = rowsum(sigma), Z = sum_k z_norm
    denom[b,s]       = rs[b,s] * Z[b]
    out[b,s,v]       = VS[b,v]                    (exactly; query cancels)

So the kernel is a column-sum of `value` over S; every output row b,s
is the same vector VS[b,:].  Sharding: data-parallel over batch, one
NeuronCore per batch element; each core reduces its 16.8 MB value
shard to the 4 KB row VS[b,:], and the host's unshard step broadcasts
that row over S (pure replication — no arithmetic).

Schedule per core (v11).  The 16 SDMA engines sustain ~360-430 GB/s
HBM->SBUF (2 NCs share a 716 GB/s stack; the exact rate depends on
how much the stack-neighbour core's stream overlaps), so the floor is
the 16.8 MB read (~40-47 us) plus the ~6.6 us NEFF prologue and a
~4.5 us tail (last-piece reduce chain + 4 KB out DMA + end barrier):
  - p-major input layout: partition p holds 32 CONTIGUOUS DRAM rows
    [32p, 32p+32); row placement is irrelevant (everything is summed).
  - input split across BOTH HWDGE engines (SP + Activation), rows
    0..15 / 16..31, transfers per engine of [4096, 4096, 4096, 2048,
    1024, 512, 448, 64] f32/partition: big transfers up front (fewer
    completion events), tiny at the end so the post-last-byte chain
    is as short as possible.
  - pair k = (sync row k, scalar row k+16) lands atomically; the DVE
    adds each pair into a tmp ring CASTING TO BF16, so the PE's
    PSUM-accumulating ones^T @ tmp (partition-reduce, ones is
    [128,1]) costs 1 HW pass instead of f32's 2.  bf16 pair rounding
    contributes ~1.7e-3 relative error (tolerance is 2e-2; fp32
    keeps the partition/psum accumulation exact).
  - output columns are split into segments 256/256/256/192/64, each
    accumulating in its OWN PSUM bank: PSUM dependencies are
    bank-granular, so a segment's stop+drain never blocks a later
    segment's matmul.
  - the final row-pair arrives as three pieces aligned to the
    segment grid; each stops and drains its own bank, keeping only a
    ~0.55 us add64->matmul64->copy64 chain (plus DMA dispatch) after
    the last byte.
  - DVE drains PSUM to SBUF f32; one 4 KB DMA writes VS.  No ACT
    compute op anywhere -> no activation-table load.
"""

import numpy as np

B, S, D = 8, 4096, 1024
P = 128                 # SBUF partitions
RPP = S // P            # 32 rows per partition (p-major layout)
# f32 elements/partition per transfer (per engine; 16 rows = 16384).
# Big transfers up front (fewer completion events), tiny at the end:
# the final transfer is just 64 f32/partition so the post-last-byte
# add -> matmul -> drain chain is as short as possible.
GROUPS_ELEMS = [4096, 4096, 4096, 2048, 1024, 512, 448, 64]
# Output column segments; segment i accumulates in its OWN PSUM bank,
# so no drain ever shares a bank with a still-pending matmul (PSUM
# dependencies are bank-granular).  Segment boundaries line up with
# the input-transfer boundaries of the last row (512 / 448 / 64).
SEGMENTS = [(0, 256), (256, 256), (512, 256), (768, 192), (960, 64)]
TMP_SLOTS = 4
H = 512                 # PSUM bank width in f32 (matmul N limit)

_CACHE: dict = {}


def _build_program():
    import concourse.mybir as mybir
    import concourse.tile as tile
    from concourse import bacc

    f32 = mybir.dt.float32
    bf16 = mybir.dt.bfloat16
    assert sum(GROUPS_ELEMS) == 16 * D
    assert sum(w for _, w in SEGMENTS) == D and len(SEGMENTS) <= 8
    nc = bacc.Bacc("TRN2", target_bir_lowering=False, debug=False, num_devices=B, enable_asserts=False)
    v = nc.declare_dram_parameter("value", [S, D], f32, isOutput=False)
    o = nc.declare_dram_parameter("out", [1, D], f32, isOutput=True)

    v_pm = v[:].rearrange("(p r) m -> p (r m)", p=P)       # [128][32*1024]

    with tile.TileContext(nc) as tc:
        with (
            tc.tile_pool(name="in", bufs=1) as in_pool,
            tc.tile_pool(name="tmp", bufs=1) as tmp_pool,
            tc.tile_pool(name="ones", bufs=1) as ones_pool,
            tc.tile_pool(name="warm", bufs=1) as warm_pool,
            tc.tile_pool(name="res", bufs=1) as res_pool,
            tc.tile_pool(name="psum", bufs=1, space="PSUM") as psum_pool,
        ):
            t = in_pool.tile([P, RPP * D], f32)
            tmp = tmp_pool.tile([P, TMP_SLOTS * D], bf16)
            ones = ones_pool.tile([P, 1], bf16)
            warm = warm_pool.tile([P, 1], f32)
            # Segment i of the output accumulates in PSUM bank i.
            ps = psum_pool.tile([1, len(SEGMENTS) * H], f32)

            # Input DMAs: each engine issues its transfers back-to-back.
            for half, eng in ((0, nc.sync), (1, nc.scalar)):
                e0 = half * 16 * D                   # element offset
                for g in GROUPS_ELEMS:
                    sl = slice(e0, e0 + g)
                    eng.dma_start(t[:, sl], v_pm[:, sl])
                    e0 += g

            nc.vector.memset(ones[:], 1.0)
            # ACT drains segments 0-1 at the end; pre-warm its
            # activation-table load here, hidden under the stream.
            nc.scalar.copy(warm[:], ones[:])

            # Pairs 0..14: DVE add (f32 -> bf16 tmp), PE accumulates
            # one matmul per output segment, each in its own PSUM bank
            # (partition reduce via ones[128,1], 1 HW pass in bf16).
            for k in range(15):
                a = t[:, k * D : (k + 1) * D]
                b = t[:, (k + 16) * D : (k + 17) * D]
                tk = tmp[:, (k % TMP_SLOTS) * D : (k % TMP_SLOTS + 1) * D]
                nc.vector.tensor_add(tk, a, b)
                for i, (lo, w) in enumerate(SEGMENTS):
                    nc.tensor.matmul(
                        ps[:, i * H : i * H + w],
                        ones[:],
                        tk[:, lo : lo + w],
                        start=(k == 0),
                        stop=False,
                    )

            # Pair 15 arrives as three pieces whose boundaries line up
            # with the segment grid (cols 0-511, 512-959, 960-1023).
            # Each segment stops and drains its OWN bank, so only the
            # final 256 B/partition piece's add -> matmul -> copy chain
            # is on the post-last-byte path.  Segments 0-1 drain on ACT
            # (pre-warmed above): the in-order DVE queue must stay
            # clear so the final add64 fires the moment its data's
            # completion semaphore does, not after a backlog of copies.
            res = res_pool.tile([1, D], f32)
            pieces = [(0, 512), (512, 448), (960, 64)]
            for plo, pw in pieces:
                a = t[:, 15 * D + plo : 15 * D + plo + pw]
                b = t[:, 31 * D + plo : 31 * D + plo + pw]
                th = tmp[:, 3 * D + plo : 3 * D + plo + pw]
                nc.vector.tensor_add(th, a, b)
                for i, (lo, w) in enumerate(SEGMENTS):
                    if lo < plo or lo + w > plo + pw:
                        continue
                    nc.tensor.matmul(
                        ps[:, i * H : i * H + w],
                        ones[:],
                        th[:, lo - plo : lo - plo + w],
                        start=False,
                        stop=True,
                    )
                    if i < 2:
                        nc.scalar.copy(res[:, lo : lo + w], ps[:, i * H : i * H + w])
                    else:
                        nc.vector.tensor_copy(res[:, lo : lo + w], ps[:, i * H : i * H + w])

            nc.sync.dma_start(o[:], res[:])

    nc.compile()
    return nc


def _get_program():
    if "nc" not in _CACHE:
        _CACHE["nc"] = _build_program()
    return _CACHE["nc"]


def kernel(query: np.ndarray, value: np.ndarray) -> np.ndarray:
    from concourse.bass_utils import run_bass_kernel_spmd

    del query  # output is exactly independent of query (see module docstring)
    value = np.ascontiguousarray(value, dtype=np.float32)
    assert value.shape == (B, S, D)

    nc = _get_program()
    in_maps = [{"value": value[b]} for b in range(B)]
    try:
        res = run_bass_kernel_spmd(nc, in_maps, list(range(B)))
    except Exception:
        # The tunneled runtime occasionally surfaces a transient
        # NRT_EXEC_UNIT_UNRECOVERABLE on the first dispatch; retry once.
        import time

        time.sleep(2.0)
        res = run_bass_kernel_spmd(nc, in_maps, list(range(B)))
    vs = np.stack([res.results[b]["out"].reshape(D) for b in range(B)], axis=0)
    # out[b, s, :] == VS[b, :] for every s — materialize the broadcast.
    return np.ascontiguousarray(
        np.broadcast_to(vs[:, None, :], (B, S, D)).astype(np.float32)
    )
